# revision 1
# baseline (speedup 1.0000x reference)
"""Trainium2 Bass kernel for nn_BrainRegion (liquid-gated recurrent cell).

Computes, for full inputs (B=8192, IN=H=2048):
    xin  = concat([x_t, state], -1)
    cand = tanh(xin @ Wc + state @ Uc + bc)
    gate = sigmoid(xin @ Wg + state @ Ug + bg)
    alpha = exp(-1/exp(log_step))
    h    = alpha * state + (1 - alpha) * gate * cand
    out  = layernorm(h) * gamma + beta

Strategy: data-parallel over batch across 8 NeuronCores (1024 rows/core),
weights replicated.  Algebraic fold: xin@Wc + state@Uc == x_t@Wc[:IN] +
state@(Wc[IN:] + Uc), which removes one third of the FLOPs.  Matmuls run
in bf16 with fp32 PSUM accumulation; the elementwise epilogue + layernorm
run on-device in fp32.
"""

import sys

if "/opt/trn_rl_repo" not in sys.path:
    sys.path.insert(0, "/opt/trn_rl_repo")

import numpy as np
import ml_dtypes

B, IN, H = 8192, 2048, 2048
NCORES = 8
BC = B // NCORES      # rows per core (1024)
P = 128               # partitions
G = BC // P           # batch groups per core (8)
NJ = 8                # H slices
NSL = H // NJ         # slice width (256)
KT = H // P           # k-tiles per matrix (16)
EPS = 1e-5

bf16 = ml_dtypes.bfloat16

# Set by test.py to collect a hardware profile.
TRACE = False
LAST_RESULTS = None

_compiled = {}


def _build(flags):
    """Trace + compile the SPMD device program. flags = (has_bc, has_bg,
    has_gamma, has_beta) selects optional elementwise passes."""
    from contextlib import ExitStack

    import concourse.bass as bass
    import concourse.tile as tile
    from concourse import bacc, mybir

    has_bc, has_bg, has_gamma, has_beta = flags
    f32 = mybir.dt.float32
    bft = mybir.dt.bfloat16
    AF = mybir.ActivationFunctionType
    OP = mybir.AluOpType

    nc = bacc.Bacc("TRN2", target_bir_lowering=False, debug=False,
                   num_devices=NCORES)

    # DRAM I/O. Activation tensors are pre-arranged on host so every DMA
    # below is contiguous:
    #   x4/s4:  [G, P, KT, P]   bf16, [g,p,k,m] = x[g*128+m, k*128+p]
    #   w*:     [NJ, P, KT, NSL] bf16, [j,p,k,n] = W[k*128+p, j*NSL+n]
    x4 = nc.dram_tensor("x4", [G, P, KT, P], bft, kind="ExternalInput").ap()
    s4 = nc.dram_tensor("s4", [G, P, KT, P], bft, kind="ExternalInput").ap()
    st = nc.dram_tensor("st", [BC, H], f32, kind="ExternalInput").ap()
    wcx = nc.dram_tensor("wcx", [NJ, P, KT, NSL], bft, kind="ExternalInput").ap()
    wcs = nc.dram_tensor("wcs", [NJ, P, KT, NSL], bft, kind="ExternalInput").ap()
    wgx = nc.dram_tensor("wgx", [NJ, P, KT, NSL], bft, kind="ExternalInput").ap()
    wgs = nc.dram_tensor("wgs", [NJ, P, KT, NSL], bft, kind="ExternalInput").ap()
    logb = nc.dram_tensor("logb", [P, H], f32, kind="ExternalInput").ap()
    vecs = {}
    for name, used in (("bcb", has_bc), ("bgb", has_bg),
                       ("gammab", has_gamma), ("betab", has_beta)):
        if used:
            vecs[name] = nc.dram_tensor(name, [P, H], f32,
                                        kind="ExternalInput").ap()
    out = nc.dram_tensor("out", [BC, H], f32, kind="ExternalOutput").ap()

    with tile.TileContext(nc) as tc, ExitStack() as ctx:
        singles = ctx.enter_context(tc.tile_pool(name="singles", bufs=1))
        actp = ctx.enter_context(tc.tile_pool(name="actp", bufs=1))
        wp = ctx.enter_context(tc.tile_pool(name="wp", bufs=2))
        psp = ctx.enter_context(tc.tile_pool(name="psp", bufs=2, space="PSUM"))
        epp = ctx.enter_context(tc.tile_pool(name="epp", bufs=2))
        stp = ctx.enter_context(tc.tile_pool(name="stp", bufs=2))
        hp = ctx.enter_context(tc.tile_pool(name="hp", bufs=1))
        statp = ctx.enter_context(tc.tile_pool(name="statp", bufs=1))
        normp = ctx.enter_context(tc.tile_pool(name="normp", bufs=4))
        outp = ctx.enter_context(tc.tile_pool(name="outp", bufs=2))

        # ---- j=0 weight slices first: the first matmul waits on these.
        # Chunk along k (interleaved across the 4 matrices) so the first
        # matmuls can start after ~1 MB instead of ~5 MB of DMA.
        w_names = (("wcx", wcx), ("wcs", wcs), ("wgx", wgx), ("wgs", wgs))
        wt0 = {name: wp.tile([P, KT, NSL], bft, name=f"{name}_j0", tag=name)
               for name, _ in w_names}
        xs_t = [actp.tile([P, KT, P], bft, name=f"x_g{g}", tag=f"x{g}")
                for g in range(G)]
        ss_t = [actp.tile([P, KT, P], bft, name=f"s_g{g}", tag=f"s{g}")
                for g in range(G)]
        for name, dram in w_names:
            nc.sync.dma_start(out=wt0[name][:], in_=dram[0])
        for g in range(G):
            nc.sync.dma_start(out=xs_t[g][:], in_=x4[g])
            nc.sync.dma_start(out=ss_t[g][:], in_=s4[g])

        # ---- constants: alpha = exp(-exp(-log_step)), broadcast [P, H] ----
        alpha_t = singles.tile([P, H], f32, name="alpha_t")
        nc.sync.dma_start(out=alpha_t[:], in_=logb[:])
        nc.scalar.activation(alpha_t[:], alpha_t[:], AF.Exp, scale=-1.0)
        nc.scalar.activation(alpha_t[:], alpha_t[:], AF.Exp, scale=-1.0)
        eps_t = singles.tile([P, 1], f32, name="eps_t")
        nc.vector.memset(eps_t[:], EPS)
        vt = {}
        for name in vecs:
            vt[name] = singles.tile([P, H], f32, name=name + "_t")
            nc.sync.dma_start(out=vt[name][:], in_=vecs[name][:])

        # ---- per-group h accumulator (bf16) and layernorm stats ----
        h_t = [hp.tile([P, H], bft, name=f"h_g{g}", tag=f"h{g}")
               for g in range(G)]
        stats_t = [statp.tile([P, NJ, 6], f32, name=f"stats_g{g}", tag=f"st{g}")
                   for g in range(G)]

        # ---- main loops: j = H slice, g = batch group ----
        for j in range(NJ):
            if j == 0:
                wt = wt0
            else:
                wt = {}
                for name, dram in w_names:
                    w = wp.tile([P, KT, NSL], bft, name=f"{name}_j{j}",
                                tag=name)
                    nc.sync.dma_start(out=w[:], in_=dram[j])
                    wt[name] = w
            jsl = slice(j * NSL, (j + 1) * NSL)

            for g in range(G):
                pc = psp.tile([P, NSL], f32, name=f"pc_{j}_{g}", tag="pc")
                pg = psp.tile([P, NSL], f32, name=f"pg_{j}_{g}", tag="pg")
                for k in range(KT):
                    xk = xs_t[g][:, k, :]
                    sk = ss_t[g][:, k, :]
                    nc.tensor.matmul(pc[:], xk, wt["wcx"][:, k, :],
                                     start=(k == 0), stop=False)
                    nc.tensor.matmul(pg[:], xk, wt["wgx"][:, k, :],
                                     start=(k == 0), stop=False)
                    nc.tensor.matmul(pc[:], sk, wt["wcs"][:, k, :],
                                     start=False, stop=(k == KT - 1))
                    nc.tensor.matmul(pg[:], sk, wt["wgs"][:, k, :],
                                     start=False, stop=(k == KT - 1))

                # epilogue for this (g, j) slice
                sc = epp.tile([P, NSL], f32, name=f"sc_{j}_{g}", tag="sc")
                sg = epp.tile([P, NSL], f32, name=f"sg_{j}_{g}", tag="sg")
                if has_bc:
                    nc.vector.scalar_tensor_tensor(
                        sc[:], pc[:], 1.0, vt["bcb"][:, jsl],
                        op0=OP.mult, op1=OP.add)
                    nc.scalar.activation(sc[:], sc[:], AF.Tanh)
                else:
                    nc.scalar.activation(sc[:], pc[:], AF.Tanh)
                if has_bg:
                    nc.vector.scalar_tensor_tensor(
                        sg[:], pg[:], 1.0, vt["bgb"][:, jsl],
                        op0=OP.mult, op1=OP.add)
                    nc.scalar.activation(sg[:], sg[:], AF.Sigmoid)
                else:
                    nc.scalar.activation(sg[:], pg[:], AF.Sigmoid)

                st_sl = stp.tile([P, NSL], f32, name=f"stsl_{j}_{g}", tag="stsl")
                nc.sync.dma_start(
                    out=st_sl[:],
                    in_=st[g * P:(g + 1) * P, jsl])

                # h = gc + alpha*(state - gc), with gc = gate*cand
                t2 = epp.tile([P, NSL], f32, name=f"t2_{j}_{g}", tag="t2")
                nc.vector.tensor_mul(t2[:], sc[:], sg[:])     # gate*cand
                t3 = epp.tile([P, NSL], f32, name=f"t3_{j}_{g}", tag="t3")
                nc.vector.tensor_sub(t3[:], st_sl[:], t2[:])
                nc.vector.tensor_mul(t3[:], t3[:], alpha_t[:, jsl])
                nc.vector.tensor_add(t2[:], t2[:], t3[:])

                nc.vector.bn_stats(out=stats_t[g][:, j, :], in_=t2[:])
                nc.vector.tensor_copy(out=h_t[g][:, jsl], in_=t2[:])

                if j == NJ - 1:
                    # layernorm + output for this group, overlapping the
                    # remaining groups' matmuls
                    mv = normp.tile([P, 2], f32, name=f"mv_{g}", tag="mv")
                    nc.vector.bn_aggr(out=mv[:], in_=stats_t[g][:])
                    rstd = normp.tile([P, 1], f32, name=f"rstd_{g}",
                                      tag="rstd")
                    nc.scalar.activation(rstd[:], mv[:, 1:2], AF.Sqrt,
                                         bias=eps_t[:])
                    nc.vector.reciprocal(rstd[:], rstd[:])
                    ot = outp.tile([P, H], f32, name=f"ot_{g}", tag="ot")
                    HH = H // 2
                    for half in range(2):
                        hs = slice(half * HH, (half + 1) * HH)
                        nc.vector.tensor_scalar(ot[:, hs], h_t[g][:, hs],
                                                mv[:, 0:1], rstd[:],
                                                op0=OP.subtract, op1=OP.mult)
                        if has_gamma:
                            nc.vector.tensor_mul(ot[:, hs], ot[:, hs],
                                                 vt["gammab"][:, hs])
                        if has_beta:
                            nc.vector.tensor_add(ot[:, hs], ot[:, hs],
                                                 vt["betab"][:, hs])
                        nc.sync.dma_start(out=out[g * P:(g + 1) * P, hs],
                                          in_=ot[:, hs])

    nc.compile()
    return nc


def _get_compiled(flags):
    if flags not in _compiled:
        _compiled[flags] = _build(flags)
    return _compiled[flags]


def kernel(x_t, state, Wc, Uc, bc, Wg, Ug, bg, log_step, gamma, beta):
    global LAST_RESULTS
    from concourse import bass_utils

    x_t = np.asarray(x_t, np.float32)
    state = np.asarray(state, np.float32)
    Wc = np.asarray(Wc, np.float32)
    Uc = np.asarray(Uc, np.float32)
    Wg = np.asarray(Wg, np.float32)
    Ug = np.asarray(Ug, np.float32)
    bc = np.asarray(bc, np.float32)
    bg = np.asarray(bg, np.float32)
    log_step = np.asarray(log_step, np.float32)
    gamma = np.asarray(gamma, np.float32)
    beta = np.asarray(beta, np.float32)

    # fold the recurrent weights, cast to bf16, pre-tile for the device:
    # [j, p, k, n] = W[k*128+p, j*NSL+n]
    def wtile(w):
        return np.ascontiguousarray(
            w.astype(bf16).reshape(KT, P, NJ, NSL).transpose(2, 1, 0, 3))

    w_maps = {
        "wcx": wtile(Wc[:IN]),
        "wcs": wtile(Wc[IN:] + Uc),
        "wgx": wtile(Wg[:IN]),
        "wgs": wtile(Wg[IN:] + Ug),
    }
    logb = np.ascontiguousarray(
        np.broadcast_to(log_step.reshape(1, H), (P, H)))

    flags = (bool(bc.any()), bool(bg.any()),
             bool((gamma != 1.0).any()), bool(beta.any()))
    vec_maps = {}
    if flags[0]:
        vec_maps["bcb"] = np.ascontiguousarray(
            np.broadcast_to(bc.reshape(1, H), (P, H)))
    if flags[1]:
        vec_maps["bgb"] = np.ascontiguousarray(
            np.broadcast_to(bg.reshape(1, H), (P, H)))
    if flags[2]:
        vec_maps["gammab"] = np.ascontiguousarray(
            np.broadcast_to(gamma.reshape(1, H), (P, H)))
    if flags[3]:
        vec_maps["betab"] = np.ascontiguousarray(
            np.broadcast_to(beta.reshape(1, H), (P, H)))

    nc = _get_compiled(flags)

    # per-core activation shards, pre-tiled: [g, p, k, m] = x[g*128+m, k*128+p]
    def atile(a):
        return np.ascontiguousarray(
            a.astype(bf16).reshape(G, P, KT, P).transpose(0, 3, 2, 1))

    in_maps = []
    for c in range(NCORES):
        rows = slice(c * BC, (c + 1) * BC)
        m = {
            "x4": atile(x_t[rows]),
            "s4": atile(state[rows]),
            "st": np.ascontiguousarray(state[rows]),
            "logb": logb,
        }
        m.update(w_maps)
        m.update(vec_maps)
        in_maps.append(m)

    trace_kwargs = {}
    if TRACE:
        trace_kwargs["trace_cores"] = list(range(NCORES))
    res = bass_utils.run_bass_kernel_spmd(
        nc, in_maps, core_ids=list(range(NCORES)), trace=TRACE,
        **trace_kwargs)
    LAST_RESULTS = res
    return np.concatenate([res.results[c]["out"] for c in range(NCORES)],
                          axis=0)



# revision 8
# speedup vs baseline: 1.2229x; 1.2229x over previous
"""Trainium2 Bass kernel for nn_BrainRegion (liquid-gated recurrent cell).

Computes, for full inputs (B=8192, IN=H=2048):
    xin  = concat([x_t, state], -1)
    cand = tanh(xin @ Wc + state @ Uc + bc)
    gate = sigmoid(xin @ Wg + state @ Ug + bg)
    alpha = exp(-1/exp(log_step))
    h    = alpha * state + (1 - alpha) * gate * cand
    out  = layernorm(h) * gamma + beta

Strategy: data-parallel over batch across 8 NeuronCores (1024 rows/core),
weights replicated.  Algebraic fold: xin@Wc + state@Uc == x_t@Wc[:IN] +
state@(Wc[IN:] + Uc), which removes one third of the FLOPs.  Mixed
precision on the TensorEngine: the x_t-side matmuls run in fp8 e4m3 with
perf_mode=DoubleRow (256-deep contraction per instruction, ~1.8x bf16
throughput) while the state-side matmuls run in bf16 (the folded state
weights are 3x larger in variance, so fp8 there would blow the error
budget).  Scale folding keeps the shared PSUM accumulation consistent:
x*16 and Wx*256 in fp8, Ws*4096 in bf16 (all powers of two, exact), and
the epilogue activations fold the 1/4096 back in.  Elementwise epilogue +
layernorm run on-device in fp32.
"""

import sys

if "/opt/trn_rl_repo" not in sys.path:
    sys.path.insert(0, "/opt/trn_rl_repo")

import numpy as np
import ml_dtypes

B, IN, H = 8192, 2048, 2048
NCORES = 8
BC = B // NCORES      # rows per core (1024)
P = 128               # partitions
G = BC // P           # batch groups per core (8)
GP = G // 2           # group pairs (4)
NJ = 4                # H slices per group pair
NSL = H // NJ         # slice width (512)
HH = H // 2           # output half width
K8 = H // 256         # fp8 DoubleRow k-chunks (8, 256 deep each)
K16 = H // 128        # bf16 k-chunks (16, 128 deep each)
EPS = 1e-5
SX = 16.0             # fp8 activation scale
SW8 = 256.0           # fp8 weight scale
SW16 = 4096.0         # bf16 state-weight scale (= SX*SW8, exact pow2)
RS = 1.0 / 4096.0     # epilogue rescale

bf16 = ml_dtypes.bfloat16
e4m3 = ml_dtypes.float8_e4m3

# Set by test.py to collect a hardware profile.
TRACE = False
LAST_RESULTS = None

_compiled = {}


def _build(flags):
    """Trace + compile the SPMD device program. flags = (has_bc, has_bg,
    has_gamma, has_beta) selects optional elementwise passes."""
    from contextlib import ExitStack

    import concourse.bass as bass
    import concourse.tile as tile
    from concourse import bacc, mybir

    has_bc, has_bg, has_gamma, has_beta = flags
    f32 = mybir.dt.float32
    bft = mybir.dt.bfloat16
    f8 = mybir.dt.float8e4
    AF = mybir.ActivationFunctionType
    OP = mybir.AluOpType
    DR = mybir.MatmulPerfMode.DoubleRow

    nc = bacc.Bacc("TRN2", target_bir_lowering=False, debug=False,
                   num_devices=NCORES)

    # DRAM I/O. Everything is pre-arranged on host so every DMA below is
    # contiguous (partition dim first):
    #   x8:    [G, P, K8, 2, P]   e4m3, [g,p,k,i,m] = 16*x[g*128+m, k*256+i*128+p]
    #   s16:   [G, P, K16, P]     bf16, [g,p,k,m] = s[g*128+m, k*128+p]
    #   w*x8:  [NJ, K8, P, 2, NSL] e4m3, [j,k,p,i,n] = 256*W[k*256+i*128+p, j*512+n]
    #   w*s16: [NJ, P, K16, NSL]  bf16, [j,p,k,n] = 4096*W[k*128+p, j*512+n]
    x8 = nc.dram_tensor("x8", [G, P, K8, 2, P], f8, kind="ExternalInput").ap()
    s16 = nc.dram_tensor("s16", [G, P, K16, P], bft, kind="ExternalInput").ap()
    stb = nc.dram_tensor("stb", [BC, H], bft, kind="ExternalInput").ap()
    wcx8 = nc.dram_tensor("wcx8", [NJ, K8, P, 2, NSL], f8,
                          kind="ExternalInput").ap()
    wgx8 = nc.dram_tensor("wgx8", [NJ, K8, P, 2, NSL], f8,
                          kind="ExternalInput").ap()
    wcs16 = nc.dram_tensor("wcs16", [NJ, P, K16, NSL], bft,
                           kind="ExternalInput").ap()
    wgs16 = nc.dram_tensor("wgs16", [NJ, P, K16, NSL], bft,
                           kind="ExternalInput").ap()
    logb = nc.dram_tensor("logb", [P, H], f32, kind="ExternalInput").ap()
    vecs = {}
    for name, used in (("bcb", has_bc), ("bgb", has_bg),
                       ("gammab", has_gamma), ("betab", has_beta)):
        if used:
            vecs[name] = nc.dram_tensor(name, [P, H], f32,
                                        kind="ExternalInput").ap()
    out = nc.dram_tensor("out", [BC, H], f32, kind="ExternalOutput").ap()

    with tile.TileContext(nc) as tc, ExitStack() as ctx:
        singles = ctx.enter_context(tc.tile_pool(name="singles", bufs=1))
        actp = ctx.enter_context(tc.tile_pool(name="actp", bufs=2))
        wxp = ctx.enter_context(tc.tile_pool(name="wxp", bufs=1))
        wsp = ctx.enter_context(tc.tile_pool(name="wsp", bufs=2))
        psp = ctx.enter_context(tc.tile_pool(name="psp", bufs=2, space="PSUM"))
        epp = ctx.enter_context(tc.tile_pool(name="epp", bufs=2))
        stp = ctx.enter_context(tc.tile_pool(name="stp", bufs=2))
        hp = ctx.enter_context(tc.tile_pool(name="hp", bufs=2))
        statp = ctx.enter_context(tc.tile_pool(name="statp", bufs=2))
        normp = ctx.enter_context(tc.tile_pool(name="normp", bufs=4))
        outp = ctx.enter_context(tc.tile_pool(name="outp", bufs=2))

        def load_wx(j):
            """Resident fp8 x-side weight tiles for one j slice."""
            for mat, dram in (("c", wcx8), ("g", wgx8)):
                for k in range(K8):
                    t = wxp.tile([P, 2, NSL], f8, name=f"wx{mat}_{j}_{k}",
                                 tag=f"wx{mat}_{j}_{k}")
                    nc.sync.dma_start(out=t[:], in_=dram[j, k])
                    wx_t[mat][(j, k)] = t

        def load_acts(g):
            xa = actp.tile([P, K8, 2, P], f8, name=f"x8_{g}", tag=f"x{g % 2}")
            nc.sync.dma_start(out=xa[:], in_=x8[g])
            sa = actp.tile([P, K16, P], bft, name=f"s16_{g}", tag=f"s{g % 2}")
            nc.sync.dma_start(out=sa[:], in_=s16[g])
            act_t[g] = (xa, sa)

        wx_t = {"c": {}, "g": {}}
        act_t = {}

        # ---- startup: j=0 x-weights + first group pair's activations.
        load_wx(0)
        load_acts(0)
        load_acts(1)

        # ---- constants: alpha = exp(-exp(-log_step)), broadcast [P, H] ----
        alpha_t = singles.tile([P, H], f32, name="alpha_t")
        nc.sync.dma_start(out=alpha_t[:], in_=logb[:])
        nc.scalar.activation(alpha_t[:], alpha_t[:], AF.Exp, scale=-1.0)
        nc.scalar.activation(alpha_t[:], alpha_t[:], AF.Exp, scale=-1.0)
        eps_t = singles.tile([P, 1], f32, name="eps_t")
        nc.vector.memset(eps_t[:], EPS)
        vt = {}
        for name in vecs:
            vt[name] = singles.tile([P, H], f32, name=name + "_t")
            nc.sync.dma_start(out=vt[name][:], in_=vecs[name][:])

        # ---- main loops: gp = group pair, j = H slice ----
        for gp in range(GP):
            g0, g1 = 2 * gp, 2 * gp + 1
            h_t = [hp.tile([P, H], bft, name=f"h_{g}", tag=f"h{gi}")
                   for gi, g in enumerate((g0, g1))]
            stats_t = [statp.tile([P, NJ, 6], f32, name=f"stats_{g}",
                                  tag=f"st{gi}")
                       for gi, g in enumerate((g0, g1))]

            for j in range(NJ):
                jsl = slice(j * NSL, (j + 1) * NSL)

                # streamed bf16 state-side weights for this (gp, j)
                ws_c = wsp.tile([P, K16, NSL], bft, name=f"wcs_{gp}_{j}",
                                tag="wcs")
                nc.sync.dma_start(out=ws_c[:], in_=wcs16[j])
                ws_g = wsp.tile([P, K16, NSL], bft, name=f"wgs_{gp}_{j}",
                                tag="wgs")
                nc.sync.dma_start(out=ws_g[:], in_=wgs16[j])

                st_t = []
                for gi, g in enumerate((g0, g1)):
                    t = stp.tile([P, NSL], bft, name=f"st_{gp}_{j}_{gi}",
                                 tag=f"st{gi}")
                    nc.sync.dma_start(out=t[:],
                                      in_=stb[g * P:(g + 1) * P, jsl])
                    st_t.append(t)

                pc = [psp.tile([P, NSL], f32, name=f"pc_{gp}_{j}_{gi}",
                               tag=f"pc{gi}") for gi in range(2)]
                pg = [psp.tile([P, NSL], f32, name=f"pg_{gp}_{j}_{gi}",
                               tag=f"pg{gi}") for gi in range(2)]

                # fp8 DoubleRow x-side accumulation (k = 256 per MM)
                for k in range(K8):
                    for gi, g in enumerate((g0, g1)):
                        xk = act_t[g][0][:, k]
                        nc.tensor.matmul(pc[gi][:], xk, wx_t["c"][(j, k)][:],
                                         start=(k == 0), stop=False,
                                         perf_mode=DR)
                        nc.tensor.matmul(pg[gi][:], xk, wx_t["g"][(j, k)][:],
                                         start=(k == 0), stop=False,
                                         perf_mode=DR)
                # bf16 state-side accumulation (k = 128 per MM)
                for k in range(K16):
                    for gi, g in enumerate((g0, g1)):
                        sk = act_t[g][1][:, k, :]
                        nc.tensor.matmul(pc[gi][:], sk, ws_c[:, k, :],
                                         start=False, stop=(k == K16 - 1))
                        nc.tensor.matmul(pg[gi][:], sk, ws_g[:, k, :],
                                         start=False, stop=(k == K16 - 1))

                # staggered prefetches (queue behind this phase's weights)
                if gp == 0 and j + 1 < NJ:
                    load_wx(j + 1)
                if j == 1 and gp + 1 < GP:
                    load_acts(g0 + 2)
                    load_acts(g1 + 2)

                # epilogue for this (gp, j) slice
                for gi, g in enumerate((g0, g1)):
                    sc = epp.tile([P, NSL], f32, name=f"sc_{gp}_{j}_{gi}",
                                  tag="sc")
                    sg = epp.tile([P, NSL], f32, name=f"sg_{gp}_{j}_{gi}",
                                  tag="sg")
                    if has_bc:
                        nc.vector.scalar_tensor_tensor(
                            sc[:], pc[gi][:], RS, vt["bcb"][:, jsl],
                            op0=OP.mult, op1=OP.add)
                        nc.scalar.activation(sc[:], sc[:], AF.Tanh)
                    else:
                        nc.scalar.activation(sc[:], pc[gi][:], AF.Tanh,
                                             scale=RS)
                    if has_bg:
                        nc.vector.scalar_tensor_tensor(
                            sg[:], pg[gi][:], RS, vt["bgb"][:, jsl],
                            op0=OP.mult, op1=OP.add)
                        nc.scalar.activation(sg[:], sg[:], AF.Sigmoid)
                    else:
                        nc.scalar.activation(sg[:], pg[gi][:], AF.Sigmoid,
                                             scale=RS)

                    # h = gc + alpha*(state - gc), with gc = gate*cand
                    t2 = epp.tile([P, NSL], f32, name=f"t2_{gp}_{j}_{gi}",
                                  tag="t2")
                    nc.vector.tensor_mul(t2[:], sc[:], sg[:])
                    t3 = epp.tile([P, NSL], f32, name=f"t3_{gp}_{j}_{gi}",
                                  tag="t3")
                    nc.vector.tensor_sub(t3[:], st_t[gi][:], t2[:])
                    nc.vector.tensor_mul(t3[:], t3[:], alpha_t[:, jsl])
                    nc.vector.tensor_add(t2[:], t2[:], t3[:])

                    nc.vector.bn_stats(out=stats_t[gi][:, j, :], in_=t2[:])
                    nc.vector.tensor_copy(out=h_t[gi][:, jsl], in_=t2[:])

                    if j == NJ - 1:
                        # layernorm + output for this group, overlapping the
                        # next phases' matmuls
                        mv = normp.tile([P, 2], f32, name=f"mv_{g}", tag="mv")
                        nc.vector.bn_aggr(out=mv[:], in_=stats_t[gi][:])
                        rstd = normp.tile([P, 1], f32, name=f"rstd_{g}",
                                          tag="rstd")
                        nc.scalar.activation(rstd[:], mv[:, 1:2], AF.Sqrt,
                                             bias=eps_t[:])
                        nc.vector.reciprocal(rstd[:], rstd[:])
                        for q in range(NJ):
                            hs = slice(q * NSL, (q + 1) * NSL)
                            ot = outp.tile([P, NSL], f32,
                                           name=f"ot_{g}_{q}", tag="ot")
                            nc.vector.tensor_scalar(ot[:], h_t[gi][:, hs],
                                                    mv[:, 0:1], rstd[:],
                                                    op0=OP.subtract,
                                                    op1=OP.mult)
                            if has_gamma:
                                nc.vector.tensor_mul(ot[:], ot[:],
                                                     vt["gammab"][:, hs])
                            if has_beta:
                                nc.vector.tensor_add(ot[:], ot[:],
                                                     vt["betab"][:, hs])
                            nc.sync.dma_start(out=out[g * P:(g + 1) * P, hs],
                                              in_=ot[:])

    nc.compile()
    return nc


def _get_compiled(flags):
    if flags not in _compiled:
        _compiled[flags] = _build(flags)
    return _compiled[flags]


def kernel(x_t, state, Wc, Uc, bc, Wg, Ug, bg, log_step, gamma, beta):
    global LAST_RESULTS
    from concourse import bass_utils

    x_t = np.asarray(x_t, np.float32)
    state = np.asarray(state, np.float32)
    Wc = np.asarray(Wc, np.float32)
    Uc = np.asarray(Uc, np.float32)
    Wg = np.asarray(Wg, np.float32)
    Ug = np.asarray(Ug, np.float32)
    bc = np.asarray(bc, np.float32)
    bg = np.asarray(bg, np.float32)
    log_step = np.asarray(log_step, np.float32)
    gamma = np.asarray(gamma, np.float32)
    beta = np.asarray(beta, np.float32)

    # fold the recurrent weights and pre-tile for the device
    def wx8tile(w):  # [j,k,p,i,n] = 256*W[k*256+i*128+p, j*512+n], e4m3
        a = np.clip(w * SW8, -240.0, 240.0).astype(e4m3)
        return np.ascontiguousarray(
            a.reshape(K8, 2, P, NJ, NSL).transpose(3, 0, 2, 1, 4))

    def ws16tile(w):  # [j,p,k,n] = 4096*W[k*128+p, j*512+n], bf16
        a = (w * SW16).astype(bf16)
        return np.ascontiguousarray(
            a.reshape(K16, P, NJ, NSL).transpose(2, 1, 0, 3))

    w_maps = {
        "wcx8": wx8tile(Wc[:IN]),
        "wcs16": ws16tile(Wc[IN:] + Uc),
        "wgx8": wx8tile(Wg[:IN]),
        "wgs16": ws16tile(Wg[IN:] + Ug),
    }
    logb = np.ascontiguousarray(
        np.broadcast_to(log_step.reshape(1, H), (P, H)))

    flags = (bool(bc.any()), bool(bg.any()),
             bool((gamma != 1.0).any()), bool(beta.any()))
    vec_maps = {}
    if flags[0]:
        vec_maps["bcb"] = np.ascontiguousarray(
            np.broadcast_to(bc.reshape(1, H), (P, H)))
    if flags[1]:
        vec_maps["bgb"] = np.ascontiguousarray(
            np.broadcast_to(bg.reshape(1, H), (P, H)))
    if flags[2]:
        vec_maps["gammab"] = np.ascontiguousarray(
            np.broadcast_to(gamma.reshape(1, H), (P, H)))
    if flags[3]:
        vec_maps["betab"] = np.ascontiguousarray(
            np.broadcast_to(beta.reshape(1, H), (P, H)))

    nc = _get_compiled(flags)

    # per-core activation shards, pre-tiled
    def x8tile(a):  # [g,p,k,i,m] = 16*x[g*128+m, k*256+i*128+p], e4m3
        q = np.clip(a * SX, -240.0, 240.0).astype(e4m3)
        return np.ascontiguousarray(
            q.reshape(G, P, K8, 2, P).transpose(0, 4, 2, 3, 1))

    def s16tile(a):  # [g,p,k,m] = s[g*128+m, k*128+p], bf16
        return np.ascontiguousarray(
            a.astype(bf16).reshape(G, P, K16, P).transpose(0, 3, 2, 1))

    in_maps = []
    for c in range(NCORES):
        rows = slice(c * BC, (c + 1) * BC)
        m = {
            "x8": x8tile(x_t[rows]),
            "s16": s16tile(state[rows]),
            "stb": np.ascontiguousarray(state[rows].astype(bf16)),
            "logb": logb,
        }
        m.update(w_maps)
        m.update(vec_maps)
        in_maps.append(m)

    trace_kwargs = {}
    if TRACE:
        trace_kwargs["trace_cores"] = list(range(NCORES))
    res = bass_utils.run_bass_kernel_spmd(
        nc, in_maps, core_ids=list(range(NCORES)), trace=TRACE,
        **trace_kwargs)
    LAST_RESULTS = res
    return np.concatenate([res.results[c]["out"] for c in range(NCORES)],
                          axis=0)


# revision 9
# speedup vs baseline: 1.2893x; 1.0543x over previous
"""Trainium2 Bass kernel for nn_BrainRegion (liquid-gated recurrent cell).

Computes, for full inputs (B=8192, IN=H=2048):
    xin  = concat([x_t, state], -1)
    cand = tanh(xin @ Wc + state @ Uc + bc)
    gate = sigmoid(xin @ Wg + state @ Ug + bg)
    alpha = exp(-1/exp(log_step))
    h    = alpha * state + (1 - alpha) * gate * cand
    out  = layernorm(h) * gamma + beta

Strategy: data-parallel over batch across 8 NeuronCores (1024 rows/core),
weights replicated.  Algebraic fold: xin@Wc + state@Uc == x_t@Wc[:IN] +
state@(Wc[IN:] + Uc), which removes one third of the FLOPs.  Mixed
precision on the TensorEngine: the x_t-side matmuls run in fp8 e4m3 with
perf_mode=DoubleRow (256-deep contraction per instruction, issuing at the
same 216ns as a 512-wide bf16 matmul = 2x throughput) while the
state-side matmuls run in bf16 (the folded state weights are 3x larger
in variance, so fp8 there would blow the error budget).  Scale folding
keeps the shared PSUM accumulation consistent: x*16 and Wx*256 in fp8,
Ws*4096 in bf16 (all powers of two, exact); the epilogue activations
fold 1/4096 back in.  Loop order is j-outer / group-inner so every
weight byte is DMAed exactly once (44 MB/core total traffic).
"""

import sys

if "/opt/trn_rl_repo" not in sys.path:
    sys.path.insert(0, "/opt/trn_rl_repo")

import numpy as np
import ml_dtypes

B, IN, H = 8192, 2048, 2048
NCORES = 8
BC = B // NCORES      # rows per core (1024)
P = 128               # partitions
G = BC // P           # batch groups per core (8)
GP = G // 2           # group pairs (4)
NJ = 4                # H slices
NSL = H // NJ         # slice width (512)
K8 = H // 256         # fp8 DoubleRow k-chunks (8, 256 deep each)
K16 = H // 128        # bf16 k-chunks (16, 128 deep each)
KH = K16 // 2         # k-chunks per half weight tile
EPS = 1e-5
SX = 16.0             # fp8 activation scale
SW8 = 256.0           # fp8 weight scale
SW16 = 4096.0         # bf16 state-weight scale (= SX*SW8, exact pow2)
RS = 1.0 / 4096.0     # epilogue rescale

bf16 = ml_dtypes.bfloat16
e4m3 = ml_dtypes.float8_e4m3

# Set by test.py to collect a hardware profile.
TRACE = False
LAST_RESULTS = None

_compiled = {}


def _build(flags):
    """Trace + compile the SPMD device program. flags = (has_bc, has_bg,
    has_gamma, has_beta) selects optional elementwise passes."""
    from contextlib import ExitStack

    import concourse.bass as bass
    import concourse.tile as tile
    from concourse import bacc, mybir

    has_bc, has_bg, has_gamma, has_beta = flags
    f32 = mybir.dt.float32
    bft = mybir.dt.bfloat16
    f8 = mybir.dt.float8e4
    AF = mybir.ActivationFunctionType
    OP = mybir.AluOpType
    DR = mybir.MatmulPerfMode.DoubleRow

    nc = bacc.Bacc("TRN2", target_bir_lowering=False, debug=False,
                   num_devices=NCORES)

    # DRAM I/O. Everything is pre-arranged on host so every DMA below is
    # contiguous (partition dim first):
    #   x8:    [G, P, K8, 2, P]   e4m3, [g,p,k,i,m] = 16*x[g*128+m, k*256+i*128+p]
    #   s16:   [G, P, K16, P]     bf16, [g,p,k,m] = s[g*128+m, k*128+p]
    #   w*x8:  [NJ, K8, P, 2, NSL] e4m3, [j,k,p,i,n] = 256*W[k*256+i*128+p, j*512+n]
    #   w*s16: [NJ, P, K16, NSL]  bf16, [j,p,k,n] = 4096*W[k*128+p, j*512+n]
    x8 = nc.dram_tensor("x8", [G, P, K8, 2, P], f8, kind="ExternalInput").ap()
    s16 = nc.dram_tensor("s16", [G, P, K16, P], bft, kind="ExternalInput").ap()
    stb = nc.dram_tensor("stb", [BC, H], bft, kind="ExternalInput").ap()
    wcx8 = nc.dram_tensor("wcx8", [NJ, K8, P, 2, NSL], f8,
                          kind="ExternalInput").ap()
    wgx8 = nc.dram_tensor("wgx8", [NJ, K8, P, 2, NSL], f8,
                          kind="ExternalInput").ap()
    wcs16 = nc.dram_tensor("wcs16", [NJ, P, K16, NSL], bft,
                           kind="ExternalInput").ap()
    wgs16 = nc.dram_tensor("wgs16", [NJ, P, K16, NSL], bft,
                           kind="ExternalInput").ap()
    logb = nc.dram_tensor("logb", [P, H], f32, kind="ExternalInput").ap()
    vecs = {}
    for name, used in (("bcb", has_bc), ("bgb", has_bg),
                       ("gammab", has_gamma), ("betab", has_beta)):
        if used:
            vecs[name] = nc.dram_tensor(name, [P, H], f32,
                                        kind="ExternalInput").ap()
    out = nc.dram_tensor("out", [BC, H], f32, kind="ExternalOutput").ap()

    with tile.TileContext(nc) as tc, ExitStack() as ctx:
        singles = ctx.enter_context(tc.tile_pool(name="singles", bufs=1))
        actp = ctx.enter_context(tc.tile_pool(name="actp", bufs=1))
        wxp = ctx.enter_context(tc.tile_pool(name="wxp", bufs=2))
        wsp = ctx.enter_context(tc.tile_pool(name="wsp", bufs=2))
        psp = ctx.enter_context(tc.tile_pool(name="psp", bufs=2, space="PSUM"))
        epp = ctx.enter_context(tc.tile_pool(name="epp", bufs=2))
        stp = ctx.enter_context(tc.tile_pool(name="stp", bufs=2))
        hp = ctx.enter_context(tc.tile_pool(name="hp", bufs=1))
        statp = ctx.enter_context(tc.tile_pool(name="statp", bufs=1))
        normp = ctx.enter_context(tc.tile_pool(name="normp", bufs=4))
        outp = ctx.enter_context(tc.tile_pool(name="outp", bufs=3))

        wx_t = {"c": {}, "g": {}}   # (j, k) -> tile [P, 2, NSL] f8
        ws_t = {}                   # (j, mat, half) -> tile [P, KH, NSL] bf16
        act_x = {}
        act_s = {}

        def load_wx(j):
            for mat, dram in (("c", wcx8), ("g", wgx8)):
                for k in range(K8):
                    t = wxp.tile([P, 2, NSL], f8, name=f"wx{mat}_{j}_{k}",
                                 tag=f"wx{mat}_{k}")
                    nc.sync.dma_start(out=t[:], in_=dram[j, k])
                    wx_t[mat][(j, k)] = t

        def load_ws(j):
            for mat, dram in (("c", wcs16), ("g", wgs16)):
                for hf in range(2):
                    t = wsp.tile([P, KH, NSL], bft,
                                 name=f"ws{mat}_{j}_{hf}", tag=f"ws{mat}{hf}")
                    nc.sync.dma_start(
                        out=t[:], in_=dram[j][:, hf * KH:(hf + 1) * KH, :])
                    ws_t[(j, mat, hf)] = t

        def load_x(g):
            t = actp.tile([P, K8, 2, P], f8, name=f"x8_{g}", tag=f"x{g}")
            nc.sync.dma_start(out=t[:], in_=x8[g])
            act_x[g] = t

        def load_s(g):
            t = actp.tile([P, K16, P], bft, name=f"s16_{g}", tag=f"s{g}")
            nc.sync.dma_start(out=t[:], in_=s16[g])
            act_s[g] = t

        # ---- startup DMAs, ordered to match first-phase consumption ----
        load_x(0)
        load_x(1)
        load_wx(0)
        load_s(0)
        load_s(1)
        load_ws(0)
        for g in range(2, G):
            load_x(g)
            load_s(g)

        # alpha = exp(-exp(-log_step)) as a bf16 [P, H] broadcast, computed
        # in NSL chunks through the outp ring (before any output use).
        alpha_t = singles.tile([P, H], bft, name="alpha_t")
        for q in range(NJ):
            qsl = slice(q * NSL, (q + 1) * NSL)
            t = outp.tile([P, NSL], f32, name=f"lg_{q}", tag="ot")
            nc.sync.dma_start(out=t[:], in_=logb[:, qsl])
            nc.scalar.activation(t[:], t[:], AF.Exp, scale=-1.0)
            nc.scalar.activation(alpha_t[:, qsl], t[:], AF.Exp, scale=-1.0)
        eps_t = singles.tile([P, 1], f32, name="eps_t")
        nc.vector.memset(eps_t[:], EPS)
        vt = {}
        for name in vecs:
            vt[name] = singles.tile([P, H], f32, name=name + "_t")
            nc.sync.dma_start(out=vt[name][:], in_=vecs[name][:])

        h_t = {}
        stats_t = {}

        # ---- main loops: j = H slice (outer), gp = group pair ----
        for j in range(NJ):
            jsl = slice(j * NSL, (j + 1) * NSL)
            for gp in range(GP):
                g0, g1 = 2 * gp, 2 * gp + 1
                if j == 0:
                    for g in (g0, g1):
                        h_t[g] = hp.tile([P, H], bft, name=f"h_{g}",
                                         tag=f"h{g}")
                        stats_t[g] = statp.tile([P, NJ, 6], f32,
                                                name=f"stats_{g}",
                                                tag=f"stat{g}")

                st_t = []
                for gi, g in enumerate((g0, g1)):
                    t = stp.tile([P, NSL], bft, name=f"st_{j}_{g}",
                                 tag=f"st{gi}")
                    nc.sync.dma_start(out=t[:],
                                      in_=stb[g * P:(g + 1) * P, jsl])
                    st_t.append(t)

                pc = [psp.tile([P, NSL], f32, name=f"pc_{j}_{gp}_{gi}",
                               tag=f"pc{gi}") for gi in range(2)]
                pg = [psp.tile([P, NSL], f32, name=f"pg_{j}_{gp}_{gi}",
                               tag=f"pg{gi}") for gi in range(2)]

                # fp8 DoubleRow x-side accumulation (k = 256 per MM)
                for k in range(K8):
                    for gi, g in enumerate((g0, g1)):
                        xk = act_x[g][:, k]
                        nc.tensor.matmul(pc[gi][:], xk, wx_t["c"][(j, k)][:],
                                         start=(k == 0), stop=False,
                                         perf_mode=DR)
                        nc.tensor.matmul(pg[gi][:], xk, wx_t["g"][(j, k)][:],
                                         start=(k == 0), stop=False,
                                         perf_mode=DR)
                # bf16 state-side accumulation (k = 128 per MM)
                for k in range(K16):
                    hf, kk = divmod(k, KH)
                    wc = ws_t[(j, "c", hf)]
                    wg = ws_t[(j, "g", hf)]
                    for gi, g in enumerate((g0, g1)):
                        sk = act_s[g][:, k, :]
                        nc.tensor.matmul(pc[gi][:], sk, wc[:, kk, :],
                                         start=False, stop=(k == K16 - 1))
                        nc.tensor.matmul(pg[gi][:], sk, wg[:, kk, :],
                                         start=False, stop=(k == K16 - 1))

                # prefetch next j's weights while this super-phase runs
                if gp == 0 and j + 1 < NJ:
                    load_wx(j + 1)
                    load_ws(j + 1)

                # epilogue for this (j, gp) slice
                for gi, g in enumerate((g0, g1)):
                    sc = epp.tile([P, NSL], f32, name=f"sc_{j}_{gp}_{gi}",
                                  tag="sc")
                    sg = epp.tile([P, NSL], f32, name=f"sg_{j}_{gp}_{gi}",
                                  tag="sg")
                    if has_bc:
                        nc.vector.scalar_tensor_tensor(
                            sc[:], pc[gi][:], RS, vt["bcb"][:, jsl],
                            op0=OP.mult, op1=OP.add)
                        nc.scalar.activation(sc[:], sc[:], AF.Tanh)
                    else:
                        nc.scalar.activation(sc[:], pc[gi][:], AF.Tanh,
                                             scale=RS)
                    if has_bg:
                        nc.vector.scalar_tensor_tensor(
                            sg[:], pg[gi][:], RS, vt["bgb"][:, jsl],
                            op0=OP.mult, op1=OP.add)
                        nc.scalar.activation(sg[:], sg[:], AF.Sigmoid)
                    else:
                        nc.scalar.activation(sg[:], pg[gi][:], AF.Sigmoid,
                                             scale=RS)

                    # h = gc + alpha*(state - gc), with gc = gate*cand
                    t2 = epp.tile([P, NSL], f32, name=f"t2_{j}_{gp}_{gi}",
                                  tag="t2")
                    nc.vector.tensor_mul(t2[:], sc[:], sg[:])
                    nc.vector.tensor_sub(sc[:], st_t[gi][:], t2[:])
                    nc.vector.tensor_mul(sc[:], sc[:], alpha_t[:, jsl])
                    nc.vector.tensor_add(t2[:], t2[:], sc[:])

                    nc.vector.bn_stats(out=stats_t[g][:, j, :], in_=t2[:])
                    nc.vector.tensor_copy(out=h_t[g][:, jsl], in_=t2[:])

                    if j == NJ - 1:
                        # layernorm + output for this group, overlapping the
                        # next phases' matmuls
                        mv = normp.tile([P, 2], f32, name=f"mv_{g}", tag="mv")
                        nc.vector.bn_aggr(out=mv[:], in_=stats_t[g][:])
                        rstd = normp.tile([P, 1], f32, name=f"rstd_{g}",
                                          tag="rstd")
                        nc.scalar.activation(rstd[:], mv[:, 1:2], AF.Sqrt,
                                             bias=eps_t[:])
                        nc.vector.reciprocal(rstd[:], rstd[:])
                        for q in range(NJ):
                            hs = slice(q * NSL, (q + 1) * NSL)
                            ot = outp.tile([P, NSL], f32,
                                           name=f"ot_{g}_{q}", tag="ot")
                            nc.vector.tensor_scalar(ot[:], h_t[g][:, hs],
                                                    mv[:, 0:1], rstd[:],
                                                    op0=OP.subtract,
                                                    op1=OP.mult)
                            if has_gamma:
                                nc.vector.tensor_mul(ot[:], ot[:],
                                                     vt["gammab"][:, hs])
                            if has_beta:
                                nc.vector.tensor_add(ot[:], ot[:],
                                                     vt["betab"][:, hs])
                            nc.sync.dma_start(out=out[g * P:(g + 1) * P, hs],
                                              in_=ot[:])

    nc.compile()
    return nc


def _get_compiled(flags):
    if flags not in _compiled:
        _compiled[flags] = _build(flags)
    return _compiled[flags]


def kernel(x_t, state, Wc, Uc, bc, Wg, Ug, bg, log_step, gamma, beta):
    global LAST_RESULTS
    from concourse import bass_utils

    x_t = np.asarray(x_t, np.float32)
    state = np.asarray(state, np.float32)
    Wc = np.asarray(Wc, np.float32)
    Uc = np.asarray(Uc, np.float32)
    Wg = np.asarray(Wg, np.float32)
    Ug = np.asarray(Ug, np.float32)
    bc = np.asarray(bc, np.float32)
    bg = np.asarray(bg, np.float32)
    log_step = np.asarray(log_step, np.float32)
    gamma = np.asarray(gamma, np.float32)
    beta = np.asarray(beta, np.float32)

    # fold the recurrent weights and pre-tile for the device
    def wx8tile(w):  # [j,k,p,i,n] = 256*W[k*256+i*128+p, j*512+n], e4m3
        a = np.clip(w * SW8, -240.0, 240.0).astype(e4m3)
        return np.ascontiguousarray(
            a.reshape(K8, 2, P, NJ, NSL).transpose(3, 0, 2, 1, 4))

    def ws16tile(w):  # [j,p,k,n] = 4096*W[k*128+p, j*512+n], bf16
        a = (w * SW16).astype(bf16)
        return np.ascontiguousarray(
            a.reshape(K16, P, NJ, NSL).transpose(2, 1, 0, 3))

    w_maps = {
        "wcx8": wx8tile(Wc[:IN]),
        "wcs16": ws16tile(Wc[IN:] + Uc),
        "wgx8": wx8tile(Wg[:IN]),
        "wgs16": ws16tile(Wg[IN:] + Ug),
    }
    logb = np.ascontiguousarray(
        np.broadcast_to(log_step.reshape(1, H), (P, H)))

    flags = (bool(bc.any()), bool(bg.any()),
             bool((gamma != 1.0).any()), bool(beta.any()))
    vec_maps = {}
    if flags[0]:
        vec_maps["bcb"] = np.ascontiguousarray(
            np.broadcast_to(bc.reshape(1, H), (P, H)))
    if flags[1]:
        vec_maps["bgb"] = np.ascontiguousarray(
            np.broadcast_to(bg.reshape(1, H), (P, H)))
    if flags[2]:
        vec_maps["gammab"] = np.ascontiguousarray(
            np.broadcast_to(gamma.reshape(1, H), (P, H)))
    if flags[3]:
        vec_maps["betab"] = np.ascontiguousarray(
            np.broadcast_to(beta.reshape(1, H), (P, H)))

    nc = _get_compiled(flags)

    # per-core activation shards, pre-tiled
    def x8tile(a):  # [g,p,k,i,m] = 16*x[g*128+m, k*256+i*128+p], e4m3
        q = np.clip(a * SX, -240.0, 240.0).astype(e4m3)
        return np.ascontiguousarray(
            q.reshape(G, P, K8, 2, P).transpose(0, 4, 2, 3, 1))

    def s16tile(a):  # [g,p,k,m] = s[g*128+m, k*128+p], bf16
        return np.ascontiguousarray(
            a.astype(bf16).reshape(G, P, K16, P).transpose(0, 3, 2, 1))

    in_maps = []
    for c in range(NCORES):
        rows = slice(c * BC, (c + 1) * BC)
        m = {
            "x8": x8tile(x_t[rows]),
            "s16": s16tile(state[rows]),
            "stb": np.ascontiguousarray(state[rows].astype(bf16)),
            "logb": logb,
        }
        m.update(w_maps)
        m.update(vec_maps)
        in_maps.append(m)

    trace_kwargs = {}
    if TRACE:
        trace_kwargs["trace_cores"] = list(range(NCORES))
    res = bass_utils.run_bass_kernel_spmd(
        nc, in_maps, core_ids=list(range(NCORES)), trace=TRACE,
        **trace_kwargs)
    LAST_RESULTS = res
    return np.concatenate([res.results[c]["out"] for c in range(NCORES)],
                          axis=0)


# revision 10
# speedup vs baseline: 1.4035x; 1.0886x over previous
"""Trainium2 Bass kernel for nn_BrainRegion (liquid-gated recurrent cell).

Computes, for full inputs (B=8192, IN=H=2048):
    xin  = concat([x_t, state], -1)
    cand = tanh(xin @ Wc + state @ Uc + bc)
    gate = sigmoid(xin @ Wg + state @ Ug + bg)
    alpha = exp(-1/exp(log_step))
    h    = alpha * state + (1 - alpha) * gate * cand
    out  = layernorm(h) * gamma + beta

Strategy: data-parallel over batch across 8 NeuronCores (1024 rows/core),
weights replicated.  Algebraic fold: xin@Wc + state@Uc == x_t@Wc[:IN] +
state@(Wc[IN:] + Uc), which removes one third of the FLOPs.  Mixed
precision on the TensorEngine: the x_t-side matmuls and the first 512
state channels run in fp8 e4m3 with perf_mode=DoubleRow (256-deep
contraction per instruction, issuing at the same 216ns as a 512-wide
bf16 matmul = 2x throughput); the remaining state-side channels run in
bf16 (the folded state weights are 3x larger in variance, so full fp8
there would blow the error budget).  Scale folding keeps the shared
PSUM accumulation consistent: activations*16 and W*256 in fp8,
Ws*4096 in bf16 (all powers of two, exact); the epilogue activations
fold 1/4096 back in.  Loop order is j-outer / group-inner so every
weight byte is DMAed exactly once; startup DMAs are emitted in exact
first-phase consumption order; the last j slice runs single-group
phases to minimize the post-matmul tail.
"""

import sys

if "/opt/trn_rl_repo" not in sys.path:
    sys.path.insert(0, "/opt/trn_rl_repo")

import numpy as np
import ml_dtypes

B, IN, H = 8192, 2048, 2048
NCORES = 8
BC = B // NCORES      # rows per core (1024)
P = 128               # partitions
G = BC // P           # batch groups per core (8)
GP = G // 2           # group pairs (4)
NJ = 4                # H slices
NSL = H // NJ         # slice width (512)
K8 = H // 256         # fp8 DoubleRow k-chunks on the x side (8)
SK8 = 2               # state-side fp8 DoubleRow k-chunks (first 512 ch)
SCUT = SK8 * 256      # state channels handled in fp8 (512)
K16 = (H - SCUT) // 128   # bf16 state k-chunks (12)
KQ = 4                # bf16 k-chunks per quarter weight tile
NQ = K16 // KQ        # quarter tiles (3)
EPS = 1e-5
SX = 16.0             # fp8 activation scale
SW8 = 256.0           # fp8 weight scale
SW16 = 4096.0         # bf16 state-weight scale (= SX*SW8, exact pow2)
RS = 1.0 / 4096.0     # epilogue rescale

bf16 = ml_dtypes.bfloat16
e4m3 = ml_dtypes.float8_e4m3

# Set by test.py to collect a hardware profile.
TRACE = False
LAST_RESULTS = None

_compiled = {}


def _build(flags):
    """Trace + compile the SPMD device program. flags = (has_bc, has_bg,
    has_gamma, has_beta) selects optional elementwise passes."""
    from contextlib import ExitStack

    import concourse.bass as bass
    import concourse.tile as tile
    from concourse import bacc, mybir

    has_bc, has_bg, has_gamma, has_beta = flags
    f32 = mybir.dt.float32
    bft = mybir.dt.bfloat16
    f8 = mybir.dt.float8e4
    AF = mybir.ActivationFunctionType
    OP = mybir.AluOpType
    DR = mybir.MatmulPerfMode.DoubleRow

    nc = bacc.Bacc("TRN2", target_bir_lowering=False, debug=False,
                   num_devices=NCORES)

    # DRAM I/O. Everything is pre-arranged on host so every DMA below is
    # contiguous (partition dim first):
    #   x8:    [G, P, K8, 2, P]    e4m3, [g,p,k,i,m] = 16*x[g*128+m, k*256+i*128+p]
    #   s8:    [G, P, SK8, 2, P]   e4m3, same layout for state channels < SCUT
    #   s16:   [G, P, K16, P]      bf16, [g,p,k,m] = s[g*128+m, SCUT+k*128+p]
    #   w*x8:  [NJ, K8, P, 2, NSL] e4m3, [j,k,p,i,n] = 256*W[k*256+i*128+p, j*512+n]
    #   w*s8:  [NJ, SK8, P, 2, NSL] e4m3, same for state-weight rows < SCUT
    #   w*s16: [NJ, P, K16, NSL]   bf16, [j,p,k,n] = 4096*W[SCUT+k*128+p, j*512+n]
    x8 = nc.dram_tensor("x8", [G, P, K8, 2, P], f8, kind="ExternalInput").ap()
    s8 = nc.dram_tensor("s8", [G, P, SK8, 2, P], f8,
                        kind="ExternalInput").ap()
    s16 = nc.dram_tensor("s16", [G, P, K16, P], bft,
                         kind="ExternalInput").ap()
    stb = nc.dram_tensor("stb", [BC, H], bft, kind="ExternalInput").ap()
    wcx8 = nc.dram_tensor("wcx8", [NJ, K8, P, 2, NSL], f8,
                          kind="ExternalInput").ap()
    wgx8 = nc.dram_tensor("wgx8", [NJ, K8, P, 2, NSL], f8,
                          kind="ExternalInput").ap()
    wcs8 = nc.dram_tensor("wcs8", [NJ, SK8, P, 2, NSL], f8,
                          kind="ExternalInput").ap()
    wgs8 = nc.dram_tensor("wgs8", [NJ, SK8, P, 2, NSL], f8,
                          kind="ExternalInput").ap()
    wcs16 = nc.dram_tensor("wcs16", [NJ, P, K16, NSL], bft,
                           kind="ExternalInput").ap()
    wgs16 = nc.dram_tensor("wgs16", [NJ, P, K16, NSL], bft,
                           kind="ExternalInput").ap()
    logb = nc.dram_tensor("logb", [P, H], f32, kind="ExternalInput").ap()
    vecs = {}
    for name, used in (("bcb", has_bc), ("bgb", has_bg),
                       ("gammab", has_gamma), ("betab", has_beta)):
        if used:
            vecs[name] = nc.dram_tensor(name, [P, H], f32,
                                        kind="ExternalInput").ap()
    out = nc.dram_tensor("out", [BC, H], f32, kind="ExternalOutput").ap()

    with tile.TileContext(nc) as tc, ExitStack() as ctx:
        singles = ctx.enter_context(tc.tile_pool(name="singles", bufs=1))
        actp = ctx.enter_context(tc.tile_pool(name="actp", bufs=1))
        wxp = ctx.enter_context(tc.tile_pool(name="wxp", bufs=2))
        wsp = ctx.enter_context(tc.tile_pool(name="wsp", bufs=2))
        psp = ctx.enter_context(tc.tile_pool(name="psp", bufs=2, space="PSUM"))
        epp = ctx.enter_context(tc.tile_pool(name="epp", bufs=2))
        stp = ctx.enter_context(tc.tile_pool(name="stp", bufs=2))
        hp = ctx.enter_context(tc.tile_pool(name="hp", bufs=1))
        statp = ctx.enter_context(tc.tile_pool(name="statp", bufs=1))
        normp = ctx.enter_context(tc.tile_pool(name="normp", bufs=4))
        outp = ctx.enter_context(tc.tile_pool(name="outp", bufs=3))

        wx_t = {"c": {}, "g": {}}   # (j, k) -> [P, 2, NSL] f8, x side
        wsd_t = {"c": {}, "g": {}}  # (j, k) -> [P, 2, NSL] f8, state side
        ws_t = {}                   # (j, mat, q) -> [P, KQ, NSL] bf16
        act_x = {}
        act_s8 = {}
        act_s = {}

        def load_wx(j):
            for k in range(K8):
                for mat, dram in (("c", wcx8), ("g", wgx8)):
                    t = wxp.tile([P, 2, NSL], f8, name=f"wx{mat}_{j}_{k}",
                                 tag=f"wx{mat}_{k}")
                    nc.sync.dma_start(out=t[:], in_=dram[j, k])
                    wx_t[mat][(j, k)] = t

        def load_wsd(j):
            for k in range(SK8):
                for mat, dram in (("c", wcs8), ("g", wgs8)):
                    t = wxp.tile([P, 2, NSL], f8, name=f"wsd{mat}_{j}_{k}",
                                 tag=f"wsd{mat}_{k}")
                    nc.sync.dma_start(out=t[:], in_=dram[j, k])
                    wsd_t[mat][(j, k)] = t

        def load_ws(j):
            for q in range(NQ):
                for mat, dram in (("c", wcs16), ("g", wgs16)):
                    t = wsp.tile([P, KQ, NSL], bft,
                                 name=f"ws{mat}_{j}_{q}", tag=f"ws{mat}{q}")
                    nc.sync.dma_start(
                        out=t[:], in_=dram[j][:, q * KQ:(q + 1) * KQ, :])
                    ws_t[(j, mat, q)] = t

        def load_x(g):
            t = actp.tile([P, K8, 2, P], f8, name=f"x8_{g}", tag=f"x{g}")
            nc.sync.dma_start(out=t[:], in_=x8[g])
            act_x[g] = t

        def load_s8(g):
            t = actp.tile([P, SK8, 2, P], f8, name=f"s8_{g}", tag=f"d{g}")
            nc.sync.dma_start(out=t[:], in_=s8[g])
            act_s8[g] = t

        def load_s(g):
            t = actp.tile([P, K16, P], bft, name=f"s16_{g}", tag=f"s{g}")
            nc.sync.dma_start(out=t[:], in_=s16[g])
            act_s[g] = t

        # ---- startup DMAs, ordered to match first-phase consumption ----
        load_x(0)
        load_x(1)
        load_wx(0)
        load_s8(0)
        load_s8(1)
        load_wsd(0)
        load_s(0)
        load_s(1)
        load_ws(0)
        for g in range(2, G):
            load_x(g)
            load_s8(g)
            load_s(g)

        # alpha = exp(-exp(-log_step)) as a bf16 [P, H] broadcast, computed
        # in NSL chunks through the outp ring (before any output use).
        alpha_t = singles.tile([P, H], bft, name="alpha_t")
        for q in range(NJ):
            qsl = slice(q * NSL, (q + 1) * NSL)
            t = outp.tile([P, NSL], f32, name=f"lg_{q}", tag="ot")
            nc.sync.dma_start(out=t[:], in_=logb[:, qsl])
            nc.scalar.activation(t[:], t[:], AF.Exp, scale=-1.0)
            nc.scalar.activation(alpha_t[:, qsl], t[:], AF.Exp, scale=-1.0)
        eps_t = singles.tile([P, 1], f32, name="eps_t")
        nc.vector.memset(eps_t[:], EPS)
        vt = {}
        for name in vecs:
            vt[name] = singles.tile([P, H], f32, name=name + "_t")
            nc.sync.dma_start(out=vt[name][:], in_=vecs[name][:])

        h_t = {}
        stats_t = {}

        def ensure_group_tiles(groups):
            for g in groups:
                if g not in h_t:
                    h_t[g] = hp.tile([P, H], bft, name=f"h_{g}", tag=f"h{g}")
                    stats_t[g] = statp.tile([P, NJ, 6], f32,
                                            name=f"stats_{g}", tag=f"st{g}")

        def phase(j, groups, pid):
            """One PSUM phase: full H-slice j accumulation + epilogue for
            the given batch groups (1 or 2 of them)."""
            jsl = slice(j * NSL, (j + 1) * NSL)
            ensure_group_tiles(groups)

            st_t = []
            for gi, g in enumerate(groups):
                t = stp.tile([P, NSL], bft, name=f"st_{j}_{g}",
                             tag=f"st{gi}")
                nc.sync.dma_start(out=t[:], in_=stb[g * P:(g + 1) * P, jsl])
                st_t.append(t)

            pc = [psp.tile([P, NSL], f32, name=f"pc_{j}_{pid}_{gi}",
                           tag=f"pc{gi}") for gi in range(len(groups))]
            pg = [psp.tile([P, NSL], f32, name=f"pg_{j}_{pid}_{gi}",
                           tag=f"pg{gi}") for gi in range(len(groups))]

            # fp8 DoubleRow x-side accumulation (k = 256 per MM)
            for k in range(K8):
                for gi, g in enumerate(groups):
                    xk = act_x[g][:, k]
                    nc.tensor.matmul(pc[gi][:], xk, wx_t["c"][(j, k)][:],
                                     start=(k == 0), stop=False, perf_mode=DR)
                    nc.tensor.matmul(pg[gi][:], xk, wx_t["g"][(j, k)][:],
                                     start=(k == 0), stop=False, perf_mode=DR)
            # fp8 DoubleRow state-side accumulation (first SCUT channels)
            for k in range(SK8):
                for gi, g in enumerate(groups):
                    sk = act_s8[g][:, k]
                    nc.tensor.matmul(pc[gi][:], sk, wsd_t["c"][(j, k)][:],
                                     start=False, stop=False, perf_mode=DR)
                    nc.tensor.matmul(pg[gi][:], sk, wsd_t["g"][(j, k)][:],
                                     start=False, stop=False, perf_mode=DR)
            # bf16 state-side accumulation (k = 128 per MM)
            for k in range(K16):
                q, kk = divmod(k, KQ)
                wc = ws_t[(j, "c", q)]
                wg = ws_t[(j, "g", q)]
                for gi, g in enumerate(groups):
                    sk = act_s[g][:, k, :]
                    nc.tensor.matmul(pc[gi][:], sk, wc[:, kk, :],
                                     start=False, stop=(k == K16 - 1))
                    nc.tensor.matmul(pg[gi][:], sk, wg[:, kk, :],
                                     start=False, stop=(k == K16 - 1))

            # epilogue for this (j, groups) slice
            for gi, g in enumerate(groups):
                sc = epp.tile([P, NSL], f32, name=f"sc_{j}_{pid}_{gi}",
                              tag="sc")
                sg = epp.tile([P, NSL], f32, name=f"sg_{j}_{pid}_{gi}",
                              tag="sg")
                if has_bc:
                    nc.vector.scalar_tensor_tensor(
                        sc[:], pc[gi][:], RS, vt["bcb"][:, jsl],
                        op0=OP.mult, op1=OP.add)
                    nc.scalar.activation(sc[:], sc[:], AF.Tanh)
                else:
                    nc.scalar.activation(sc[:], pc[gi][:], AF.Tanh, scale=RS)
                if has_bg:
                    nc.vector.scalar_tensor_tensor(
                        sg[:], pg[gi][:], RS, vt["bgb"][:, jsl],
                        op0=OP.mult, op1=OP.add)
                    nc.scalar.activation(sg[:], sg[:], AF.Sigmoid)
                else:
                    nc.scalar.activation(sg[:], pg[gi][:], AF.Sigmoid,
                                         scale=RS)

                # h = gc + alpha*(state - gc), with gc = gate*cand
                t2 = epp.tile([P, NSL], f32, name=f"t2_{j}_{pid}_{gi}",
                              tag="t2")
                nc.vector.tensor_mul(t2[:], sc[:], sg[:])
                nc.vector.tensor_sub(sc[:], st_t[gi][:], t2[:])
                nc.vector.tensor_mul(sc[:], sc[:], alpha_t[:, jsl])
                nc.vector.tensor_add(t2[:], t2[:], sc[:])

                nc.vector.bn_stats(out=stats_t[g][:, j, :], in_=t2[:])
                nc.vector.tensor_copy(out=h_t[g][:, jsl], in_=t2[:])

                if j == NJ - 1:
                    # layernorm + output for this group
                    mv = normp.tile([P, 2], f32, name=f"mv_{g}", tag="mv")
                    nc.vector.bn_aggr(out=mv[:], in_=stats_t[g][:])
                    rstd = normp.tile([P, 1], f32, name=f"rstd_{g}",
                                      tag="rstd")
                    nc.scalar.activation(rstd[:], mv[:, 1:2], AF.Sqrt,
                                         bias=eps_t[:])
                    nc.vector.reciprocal(rstd[:], rstd[:])
                    for q in range(NJ):
                        hs = slice(q * NSL, (q + 1) * NSL)
                        ot = outp.tile([P, NSL], f32,
                                       name=f"ot_{g}_{q}", tag="ot")
                        nc.vector.tensor_scalar(ot[:], h_t[g][:, hs],
                                                mv[:, 0:1], rstd[:],
                                                op0=OP.subtract, op1=OP.mult)
                        if has_gamma:
                            nc.vector.tensor_mul(ot[:], ot[:],
                                                 vt["gammab"][:, hs])
                        if has_beta:
                            nc.vector.tensor_add(ot[:], ot[:],
                                                 vt["betab"][:, hs])
                        nc.sync.dma_start(out=out[g * P:(g + 1) * P, hs],
                                          in_=ot[:])

        # ---- main loops: j = H slice (outer), batch groups inner.
        # Last j slice runs single-group phases to shorten the tail.
        for j in range(NJ):
            if j < NJ - 1:
                group_sets = [(2 * gp, 2 * gp + 1) for gp in range(GP)]
            else:
                group_sets = [(g,) for g in range(G)]
            for pid, groups in enumerate(group_sets):
                phase(j, groups, pid)
                # prefetch next j's weights while this super-phase runs
                if pid == 0 and j + 1 < NJ:
                    load_wx(j + 1)
                    load_wsd(j + 1)
                    load_ws(j + 1)

    nc.compile()
    return nc


def _get_compiled(flags):
    if flags not in _compiled:
        _compiled[flags] = _build(flags)
    return _compiled[flags]


def kernel(x_t, state, Wc, Uc, bc, Wg, Ug, bg, log_step, gamma, beta):
    global LAST_RESULTS
    from concourse import bass_utils

    x_t = np.asarray(x_t, np.float32)
    state = np.asarray(state, np.float32)
    Wc = np.asarray(Wc, np.float32)
    Uc = np.asarray(Uc, np.float32)
    Wg = np.asarray(Wg, np.float32)
    Ug = np.asarray(Ug, np.float32)
    bc = np.asarray(bc, np.float32)
    bg = np.asarray(bg, np.float32)
    log_step = np.asarray(log_step, np.float32)
    gamma = np.asarray(gamma, np.float32)
    beta = np.asarray(beta, np.float32)

    # fold the recurrent weights and pre-tile for the device
    def w8tile(w, nk):  # [j,k,p,i,n] = 256*W[k*256+i*128+p, j*512+n], e4m3
        a = np.clip(w * SW8, -240.0, 240.0).astype(e4m3)
        return np.ascontiguousarray(
            a.reshape(nk, 2, P, NJ, NSL).transpose(3, 0, 2, 1, 4))

    def ws16tile(w):  # [j,p,k,n] = 4096*W[k*128+p, j*512+n], bf16
        a = (w * SW16).astype(bf16)
        return np.ascontiguousarray(
            a.reshape(K16, P, NJ, NSL).transpose(2, 1, 0, 3))

    Wcs = Wc[IN:] + Uc
    Wgs = Wg[IN:] + Ug
    w_maps = {
        "wcx8": w8tile(Wc[:IN], K8),
        "wgx8": w8tile(Wg[:IN], K8),
        "wcs8": w8tile(Wcs[:SCUT], SK8),
        "wgs8": w8tile(Wgs[:SCUT], SK8),
        "wcs16": ws16tile(Wcs[SCUT:]),
        "wgs16": ws16tile(Wgs[SCUT:]),
    }
    logb = np.ascontiguousarray(
        np.broadcast_to(log_step.reshape(1, H), (P, H)))

    flags = (bool(bc.any()), bool(bg.any()),
             bool((gamma != 1.0).any()), bool(beta.any()))
    vec_maps = {}
    if flags[0]:
        vec_maps["bcb"] = np.ascontiguousarray(
            np.broadcast_to(bc.reshape(1, H), (P, H)))
    if flags[1]:
        vec_maps["bgb"] = np.ascontiguousarray(
            np.broadcast_to(bg.reshape(1, H), (P, H)))
    if flags[2]:
        vec_maps["gammab"] = np.ascontiguousarray(
            np.broadcast_to(gamma.reshape(1, H), (P, H)))
    if flags[3]:
        vec_maps["betab"] = np.ascontiguousarray(
            np.broadcast_to(beta.reshape(1, H), (P, H)))

    nc = _get_compiled(flags)

    # per-core activation shards, pre-tiled
    def a8tile(a, nk):  # [g,p,k,i,m] = 16*a[g*128+m, k*256+i*128+p], e4m3
        q = np.clip(a * SX, -240.0, 240.0).astype(e4m3)
        return np.ascontiguousarray(
            q.reshape(G, P, nk, 2, P).transpose(0, 4, 2, 3, 1))

    def s16tile(a):  # [g,p,k,m] = a[g*128+m, k*128+p], bf16
        return np.ascontiguousarray(
            a.astype(bf16).reshape(G, P, K16, P).transpose(0, 3, 2, 1))

    in_maps = []
    for c in range(NCORES):
        rows = slice(c * BC, (c + 1) * BC)
        sr = state[rows]
        m = {
            "x8": a8tile(x_t[rows], K8),
            "s8": a8tile(sr[:, :SCUT], SK8),
            "s16": s16tile(sr[:, SCUT:]),
            "stb": np.ascontiguousarray(sr.astype(bf16)),
            "logb": logb,
        }
        m.update(w_maps)
        m.update(vec_maps)
        in_maps.append(m)

    trace_kwargs = {}
    if TRACE:
        trace_kwargs["trace_cores"] = list(range(NCORES))
    res = bass_utils.run_bass_kernel_spmd(
        nc, in_maps, core_ids=list(range(NCORES)), trace=TRACE,
        **trace_kwargs)
    LAST_RESULTS = res
    return np.concatenate([res.results[c]["out"] for c in range(NCORES)],
                          axis=0)
